# revision 29
# baseline (speedup 1.0000x reference)
"""Trainium2 Bass kernel for nn_AttentionLayer (B=32, C=512, HW=1024).

Data-parallel over batch across 8 NeuronCores (4 samples each).  BN batch
stats are per-core partials + a tiny 8-core AllGather, twice (BN1 on x,
BN2 on xr = x + attention).  Attention matmuls run on TensorE in fp8
DoubleRow (fp8 Wv rounding error cancelled by a per-channel bias
dWv@mean(h), exploiting sum_q softmax == 1); the MLP runs in bf16 with
BN2 folded into W1; the residual path stays f32 and xr stays resident in
SBUF (no DRAM round-trip).  x is loaded in bf16.  BN2 stats come from
fused accumulators on the residual writes.  Dummy matmuls keep the PE
HAM clock un-throttled through the x-load and collective windows.

kernel(**inputs) takes FULL unsharded inputs, returns the FULL output.
"""

import numpy as np

B, C, HW = 32, 512, 1024
D = C // 8            # 64
N_CORES = 8
B_LOC = B // N_CORES  # 4
P = 128
CO = C // P           # 4
NTOT = float(B * HW)  # BN normalizer (biased stats over batch+spatial)
EPS = 1e-5

# PE clock-warm heartbeat (dummy matmuls, ~0.21-0.43us each)
WARM_A = 350   # covers x-load + BN1 AllGather window
WARM_2 = 130    # covers BN2 AllGather window

_CACHE = {}


def _build_nc():
    import concourse.bass as bass
    import concourse.mybir as mybir
    import concourse.tile as tile
    from concourse import bacc
    from concourse.bass import ts

    f32 = mybir.dt.float32
    bf16 = mybir.dt.bfloat16
    f8 = mybir.dt.float8e4
    PM = mybir.MatmulPerfMode
    AF = mybir.ActivationFunctionType
    ALU = mybir.AluOpType
    AX = mybir.AxisListType

    nc = bacc.Bacc("TRN2", target_bir_lowering=False, debug=False,
                   num_devices=N_CORES)

    # ---------------- I/O ----------------
    x_d = nc.dram_tensor("x", [B_LOC, C, HW], bf16, kind="ExternalInput")
    wq_d = nc.dram_tensor("wq_t", [P, CO, P], f8, kind="ExternalInput")
    wk_d = nc.dram_tensor("wk_t", [P, CO, P], f8, kind="ExternalInput")
    wv_d = nc.dram_tensor("wv_t", [P, CO, C], f8, kind="ExternalInput")
    dwv_d = nc.dram_tensor("dwv_t", [P, CO, C], bf16, kind="ExternalInput")
    w1_d = nc.dram_tensor("w1_t", [P, CO, C], bf16, kind="ExternalInput")
    w2_d = nc.dram_tensor("w2_t", [P, CO, C], bf16, kind="ExternalInput")
    bk_d = nc.dram_tensor("bk_t", [P, 1], f32, kind="ExternalInput")
    bv_d = nc.dram_tensor("bv_t", [P, CO], f32, kind="ExternalInput")
    b1_d = nc.dram_tensor("b1_t", [P, CO], f32, kind="ExternalInput")
    b2_d = nc.dram_tensor("b2_t", [P, CO], f32, kind="ExternalInput")
    g1_d = nc.dram_tensor("g1_t", [P, CO], f32, kind="ExternalInput")
    be1_d = nc.dram_tensor("be1_t", [P, CO], f32, kind="ExternalInput")
    g2_d = nc.dram_tensor("g2_t", [P, CO], f32, kind="ExternalInput")
    be2_d = nc.dram_tensor("be2_t", [P, CO], f32, kind="ExternalInput")
    ones_d = nc.dram_tensor("ones_t", [P, P], bf16, kind="ExternalInput")
    out_d = nc.dram_tensor("out", [B_LOC, C, HW], f32, kind="ExternalOutput")

    def chw_view(dram3, s):
        # [C, HW] sample -> [P, CO, HW] partition view (c = co*P + p)
        return dram3[s].rearrange("(co p) hw -> p co hw", p=P)

    with tile.TileContext(nc) as tc:
        with (
            tc.tile_pool(name="const", bufs=1) as cpool,
            tc.tile_pool(name="stats", bufs=1) as spool,
            tc.tile_pool(name="dram", bufs=1, space="DRAM") as dpool,
            tc.tile_pool(name="psum", bufs=1, space="PSUM") as ppool,
        ):
            # ---------- persistent weights ----------
            wq = cpool.tile([P, CO, P], f8)
            wk = cpool.tile([P, CO, P], f8)
            wv = cpool.tile([P, CO, C], f8)
            dwv = cpool.tile([P, CO, C], bf16)
            w1 = cpool.tile([P, CO, C], bf16)
            w1a2 = cpool.tile([P, CO, C], bf16)   # W1 * a2 (BN2 folded)
            w2 = cpool.tile([P, CO, C], bf16)
            bk = cpool.tile([P, 1], f32)
            bv = cpool.tile([P, CO], f32)
            b1 = cpool.tile([P, CO], f32)
            b1p = cpool.tile([P, CO], f32)        # b1 + W1 @ d2
            b2 = cpool.tile([P, CO], f32)
            g1 = cpool.tile([P, CO], f32)
            be1 = cpool.tile([P, CO], f32)
            g2 = cpool.tile([P, CO], f32)
            be2 = cpool.tile([P, CO], f32)
            d2b = cpool.tile([P, CO], bf16)
            ones128 = cpool.tile([P, P], bf16)
            dummy = cpool.tile([P, 512], bf16)
            eps_t = cpool.tile([P, 1], f32)
            nc.gpsimd.memset(eps_t[:], EPS)
            nc.gpsimd.memset(dummy[:], 0.5)

            # ---------- stats tiles ----------
            ssum1 = spool.tile([P, CO, B_LOC], f32)
            ssq1 = spool.tile([P, CO, B_LOC], f32)
            ssum2 = spool.tile([P, CO, B_LOC], f32)
            s2acc = spool.tile([P, CO, 2 * B_LOC], f32)   # attsum per (mo,n2)
            q2acc = spool.tile([P, CO, 2 * B_LOC], f32)   # sum(xr^2) per (mo,n2)
            ccin1 = spool.tile([P, 2 * CO], f32)
            ccin2 = spool.tile([P, 2 * CO], f32)
            ag1 = spool.tile([P, N_CORES, 2 * CO], f32)
            ag2 = spool.tile([P, N_CORES, 2 * CO], f32)
            a1 = spool.tile([P, CO], f32)
            d1 = spool.tile([P, CO], f32)
            a2 = spool.tile([P, CO], f32)
            d2 = spool.tile([P, CO], f32)
            mtmp = spool.tile([P, CO], f32)
            vtmp = spool.tile([P, CO], f32)
            ttmp = spool.tile([P, CO], f32)
            agt = spool.tile([P, 2 * CO], f32)
            junk1 = spool.tile([P, 1], f32)

            # DRAM scratch (collective in/out only)
            cc1i_d = dpool.tile([P, 2 * CO], f32)
            cc1o_d = dpool.tile([N_CORES * P, 2 * CO], f32)
            cc2i_d = dpool.tile([P, 2 * CO], f32)
            cc2o_d = dpool.tile([N_CORES * P, 2 * CO], f32)

            def heartbeat(n):
                """Dummy matmuls keeping the PE HAM clock at 8/8 through
                windows where real matmuls are blocked on collectives."""
                if n <= 0:
                    return
                wt = ppool.tile([P, 512], f32, tag="psC", bufs=1)
                for _ in range(n):
                    nc.tensor.matmul(wt[:], dummy[:, 0:P], dummy[:],
                                     start=True, stop=True)

            def bn_coeffs(cci_d, cco_d, ag_sb, gg, bb, aa, dd):
                """AllGather -> local sum -> a = g*rsqrt(var+eps),
                d = b - mean*a"""
                nc.gpsimd.collective_compute(
                    "AllGather", ALU.bypass,
                    replica_groups=[list(range(N_CORES))],
                    ins=[cci_d[:].opt()], outs=[cco_d[:].opt()],
                )
                nc.scalar.dma_start(
                    ag_sb[:],
                    cco_d[:].rearrange("(r p) f -> p r f", p=P))
                nc.vector.tensor_add(agt[:], ag_sb[:, 0, :], ag_sb[:, 1, :])
                for rr in range(2, N_CORES):
                    nc.vector.tensor_add(agt[:], agt[:], ag_sb[:, rr, :])
                nc.vector.tensor_scalar_mul(mtmp[:], agt[:, 0:CO],
                                            1.0 / NTOT)
                nc.vector.tensor_scalar_mul(vtmp[:], agt[:, CO:2 * CO],
                                            1.0 / NTOT)
                nc.vector.tensor_mul(ttmp[:], mtmp[:], mtmp[:])
                nc.vector.tensor_sub(vtmp[:], vtmp[:], ttmp[:])
                nc.scalar.activation(vtmp[:], vtmp[:], AF.Sqrt, bias=eps_t[:])
                nc.vector.reciprocal(ttmp[:], vtmp[:])
                nc.vector.tensor_mul(aa[:], gg[:], ttmp[:])
                nc.vector.tensor_mul(ttmp[:], mtmp[:], aa[:])
                nc.vector.tensor_sub(dd[:], bb[:], ttmp[:])

            with tc.tile_pool(name="xrp", bufs=1) as xrpool:
                xr_all = xrpool.tile([P, B_LOC, CO, HW], f32)

                with tc.tile_pool(name="xp", bufs=1) as xpool:
                    x_all = xpool.tile([P, B_LOC, CO, HW], bf16)

                    # PE heartbeat through x-load + AG1 (independent ops,
                    # run back-to-back from t~0)
                    heartbeat(WARM_A)

                    # ============ pass 1: x load + BN1 stats ============
                    # 32 reduction passes spread across DVE/ACT/Pool so no
                    # single engine trails the DMA stream
                    with tc.tile_pool(name="p1", bufs=2) as w1pool:
                        for s in range(B_LOC):
                            for co in range(CO):
                                nc.sync.dma_start(
                                    x_all[:, s, co:co + 1, :],
                                    chw_view(x_d, s)[:, co:co + 1, :])
                                i = 4 * s + co
                                xt_a = x_all[:, s, co, :]
                                # sums: 12 DVE, 4 ACT / squares: 12 ACT,
                                # 4 DVE -> 16 passes each engine
                                sq = w1pool.tile([P, HW], bf16, tag="sq1")
                                if i % 4 == 3:
                                    pj = w1pool.tile([P, HW], bf16,
                                                     tag="pj1")
                                    nc.scalar.activation(
                                        pj[:], xt_a, AF.Identity,
                                        accum_out=ssum1[:, co, s:s + 1])
                                    nc.vector.affine_mul_reduce(
                                        out=sq[:],
                                        accum_out=ssq1[:, co, s:s + 1],
                                        in0=xt_a, in1=xt_a,
                                        scale=1.0, bias=0.0)
                                else:
                                    nc.vector.tensor_reduce(
                                        ssum1[:, co, s:s + 1], xt_a,
                                        axis=AX.X, op=ALU.add)
                                    nc.scalar.activation(
                                        sq[:], xt_a, AF.Square,
                                        accum_out=ssq1[:, co, s:s + 1])

                    # weight/bias loads (issued after the x DMAs on purpose)
                    for t, d in [(wq, wq_d), (wk, wk_d), (wv, wv_d),
                                 (dwv, dwv_d), (w1, w1_d),
                                 (w2, w2_d), (bk, bk_d), (bv, bv_d),
                                 (b1, b1_d), (b2, b2_d), (g1, g1_d),
                                 (be1, be1_d), (g2, g2_d), (be2, be2_d),
                                 (ones128, ones_d)]:
                        nc.sync.dma_start(t[:], d[:])

                    nc.vector.tensor_reduce(ccin1[:, 0:CO, None], ssum1[:],
                                            axis=AX.X, op=ALU.add)
                    nc.scalar.dma_start(cc1i_d[:, 0:CO], ccin1[:, 0:CO])
                    nc.vector.tensor_reduce(ccin1[:, CO:2 * CO, None],
                                            ssq1[:], axis=AX.X, op=ALU.add)
                    nc.scalar.dma_start(cc1i_d[:, CO:2 * CO],
                                        ccin1[:, CO:2 * CO])
                    bn_coeffs(cc1i_d, cc1o_d, ag1, g1, be1, a1, d1)

                    # ======== pass 2: attention, xr = x + att ========
                    with tc.tile_pool(name="p2b", bufs=2) as bpool:
                        for s in range(B_LOC):
                            if s > 0:
                                # cover the PE idle window while ACT
                                # computes this sample's h (HAM would
                                # re-throttle after ~3.4us idle)
                                heartbeat(8)
                            xt = x_all[:, s]
                            qz = bpool.tile([P, HW], bf16, tag="qz")
                            kz = bpool.tile([P, HW], bf16, tag="kz")

                            # h = relu(a1*x + d1); hsum rows for the fp8-Wv
                            # DC correction (sum_q E/Z == 1 exactly)
                            h = bpool.tile([P, CO, HW], f8, tag="h", bufs=2)
                            hsum = bpool.tile([P, CO], f32, tag="hsum")
                            for co in range(CO):
                                nc.scalar.activation(
                                    h[:, co, :], xt[:, co, :], AF.Relu,
                                    bias=d1[:, co:co + 1],
                                    scale=a1[:, co:co + 1],
                                    accum_out=hsum[:, co:co + 1])
                            # q = Wq @ h (bias dropped: constant-per-column
                            # terms cancel in softmax over q), k = Wk @ h +
                            # bk; each duplicated into both partition halves
                            # so the beta matmuls can row-pack two K=64 tiles
                            for n2 in range(2):
                                qps = ppool.tile([P, 512], f32, tag="ps512",
                                                 bufs=7)
                                for c2 in range(2):
                                    nc.tensor.matmul(
                                        qps[:],
                                        wq[:, 2 * c2:2 * c2 + 2, :],
                                        h[:, 2 * c2:2 * c2 + 2, ts(n2, 512)],
                                        start=(c2 == 0), stop=(c2 == 1),
                                        perf_mode=PM.DoubleRow)
                                nc.scalar.activation(qz[:, ts(n2, 512)],
                                                     qps[:], AF.Identity)
                                kps = ppool.tile([P, 512], f32, tag="ps512",
                                                 bufs=7)
                                for c2 in range(2):
                                    nc.tensor.matmul(
                                        kps[:],
                                        wk[:, 2 * c2:2 * c2 + 2, :],
                                        h[:, 2 * c2:2 * c2 + 2, ts(n2, 512)],
                                        start=(c2 == 0), stop=(c2 == 1),
                                        perf_mode=PM.DoubleRow)
                                nc.scalar.activation(kz[:, ts(n2, 512)],
                                                     kps[:], AF.Identity,
                                                     bias=bk[:])

                            # vT[hw, c] = h^T @ Wv^T (bv folded into xr)
                            vt = bpool.tile([P, 8, C], f8, tag="vt", bufs=2)
                            for jw in range(8):
                                vtps = ppool.tile([P, 512], f32, tag="ps512",
                                                  bufs=7)
                                for c2 in range(2):
                                    nc.tensor.matmul(
                                        vtps[:],
                                        h[:, 2 * c2:2 * c2 + 2, ts(jw, P)],
                                        wv[:, 2 * c2:2 * c2 + 2, :],
                                        start=(c2 == 0), stop=(c2 == 1),
                                        perf_mode=PM.DoubleRow)
                                if jw % 2 == 0:
                                    nc.vector.tensor_copy(vt[:, jw, :],
                                                          vtps[:])
                                else:
                                    nc.scalar.activation(vt[:, jw, :],
                                                         vtps[:],
                                                         AF.Identity)

                            # E = exp(q^T k / 8) in [q, k] layout, with a
                            # bf16 tree presum for Z on the idle Pool engine
                            E = bpool.tile([P, 8, HW], f8, tag="E", bufs=2)
                            et = bpool.tile([P, 4, HW], bf16, tag="et",
                                            bufs=1)
                            lo, hi = slice(0, D), slice(D, P)
                            for j2 in range(4):
                                je, jo = 2 * j2, 2 * j2 + 1
                                bps = {}
                                for n2 in range(2):
                                    be = ppool.tile([P, 512], f32,
                                                    tag="ps512", bufs=7)
                                    bo = ppool.tile([P, 512], f32,
                                                    tag="ps512", bufs=7)
                                    nc.tensor.matmul(be[:],
                                                     qz[lo, ts(je, P)],
                                                     kz[lo, ts(n2, 512)],
                                                     start=True, stop=True)
                                    nc.tensor.matmul(bo[:],
                                                     qz[hi, ts(jo, P)],
                                                     kz[hi, ts(n2, 512)],
                                                     start=True, stop=True)
                                    bps[n2] = (be, bo)
                                for n2 in range(2):
                                    be, bo = bps[n2]
                                    nc.scalar.activation(
                                        E[:, je, ts(n2, 512)],
                                        be[:], AF.Exp, scale=0.125)
                                    nc.scalar.activation(
                                        E[:, jo, ts(n2, 512)],
                                        bo[:], AF.Exp, scale=0.125)
                                nc.vector.tensor_add(et[:, j2, :],
                                                     E[:, je, :],
                                                     E[:, jo, :])

                            # fp8-Wv DC correction bias
                            hm = bpool.tile([P, CO], bf16, tag="hm")
                            nc.vector.tensor_scalar_mul(hm[:], hsum[:],
                                                        1.0 / HW)
                            cps = ppool.tile([P, 512], f32, tag="psC",
                                             bufs=1)
                            for mo in range(CO):
                                for ci in range(CO):
                                    nc.tensor.matmul(cps[:, mo:mo + 1],
                                                     dwv[:, ci, ts(mo, P)],
                                                     hm[:, ci, None],
                                                     start=(ci == 0),
                                                     stop=(ci == 3))
                            biasn = bpool.tile([P, CO], f32, tag="biasn")
                            nc.vector.tensor_add(biasn[:], cps[:, 0:CO],
                                                 bv[:])

                            # att = (v @ E) / Z ; xr = x + att + bias
                            aps_tiles = {}
                            rz = bpool.tile([P, HW], f32, tag="rz")

                            # Z partition-reduce + reciprocal (before the
                            # att groups so rz never gates a consume)
                            for n2 in range(2):
                                zps = ppool.tile([P, 512], f32,
                                                 tag="ps512", bufs=7)
                                for j2 in range(4):
                                    nc.tensor.matmul(
                                        zps[:],
                                        ones128[:],
                                        et[:, j2, ts(n2, 512)],
                                        start=(j2 == 0),
                                        stop=(j2 == 3))
                                nc.vector.reciprocal_approx_fast(
                                    out=rz[:, ts(n2, 512)],
                                    in_=zps[:])

                            def att_group(mo, n2):
                                aps = ppool.tile([P, 512], f32, tag="ps512",
                                                 bufs=7)
                                for j4 in range(4):
                                    nc.tensor.matmul(
                                        aps[:],
                                        vt[:, 2 * j4:2 * j4 + 2, ts(mo, P)],
                                        E[:, 2 * j4:2 * j4 + 2, ts(n2, 512)],
                                        start=(j4 == 0), stop=(j4 == 3),
                                        perf_mode=PM.DoubleRow)
                                aps_tiles[(mo, n2)] = aps

                            last_s = (s == B_LOC - 1)

                            def consume(mo, n2):
                                aps = aps_tiles.pop((mo, n2))
                                dst = xr_all[:, s, mo, ts(n2, 512)]
                                i2 = 2 * s + n2
                                # att = aps*rz (accum: attsum), then
                                # xr = att + biasn + x, then sumsq accum
                                # via a square affine_mul_reduce
                                nc.vector.affine_mul_reduce(
                                    out=dst,
                                    accum_out=s2acc[:, mo, i2:i2 + 1],
                                    in0=aps[:], in1=rz[:, ts(n2, 512)],
                                    scale=1.0, bias=0.0)
                                nc.vector.affine_then_add(
                                    out=dst, in0=dst,
                                    in1=xt[:, mo, ts(n2, 512)],
                                    scale=1.0, bias=biasn[:, mo:mo + 1])
                                sqj = bpool.tile([P, 512], bf16, tag="sqj",
                                                 bufs=2)
                                nc.vector.affine_mul_reduce(
                                    out=sqj[:],
                                    accum_out=q2acc[:, mo, i2:i2 + 1],
                                    in0=dst, in1=dst,
                                    scale=1.0, bias=0.0)

                            groups = [(mo, n2) for mo in range(CO)
                                      for n2 in range(2)]
                            for idx, g in enumerate(groups):
                                att_group(*g)
                                lag = 2 if last_s else 5
                                if idx >= lag:
                                    consume(*groups[idx - lag])
                            for g in groups[-(2 if last_s else 5):]:
                                consume(*g)

                            # ssum2[:, :, s] = sum_hw(x) + attsum + HW*bias
                            atot = bpool.tile([P, CO], f32, tag="atot")
                            nc.vector.tensor_reduce(
                                atot[:, :, None],
                                s2acc[:, :, 2 * s:2 * s + 2],
                                axis=AX.X, op=ALU.add)
                            nc.vector.tensor_add(atot[:], atot[:],
                                                 ssum1[:, :, s])
                            nc.vector.tensor_scalar(ssum2[:, :, s],
                                                    biasn[:],
                                                    float(HW), None,
                                                    ALU.mult, ALU.bypass)
                            nc.vector.tensor_add(ssum2[:, :, s],
                                                 ssum2[:, :, s], atot[:])

                        # pack BN2 partials -> AG2
                        nc.vector.tensor_reduce(ccin2[:, 0:CO, None],
                                                ssum2[:], axis=AX.X,
                                                op=ALU.add)
                        nc.scalar.dma_start(cc2i_d[:, 0:CO],
                                            ccin2[:, 0:CO])
                        nc.vector.tensor_reduce(ccin2[:, CO:2 * CO, None],
                                                q2acc[:], axis=AX.X,
                                                op=ALU.add)
                        nc.scalar.dma_start(cc2i_d[:, CO:2 * CO],
                                            ccin2[:, CO:2 * CO])
                        nc.gpsimd.collective_compute(
                            "AllGather", ALU.bypass,
                            replica_groups=[list(range(N_CORES))],
                            ins=[cc2i_d[:].opt()], outs=[cc2o_d[:].opt()],
                        )

                # ===== gap3: keep PE warm; cast xr->bf16 during AG2 =====
                heartbeat(WARM_2)

                with tc.tile_pool(name="mp", bufs=2) as mpool:
                    xrb = mpool.tile([P, B_LOC, CO, HW], bf16, tag="xrb",
                                     bufs=1)
                    for s in range(B_LOC):
                        for co in range(CO):
                            if (4 * s + co) % 2 == 0:
                                nc.vector.tensor_copy(xrb[:, s, co, :],
                                                      xr_all[:, s, co, :])
                            else:
                                nc.scalar.activation(xrb[:, s, co, :],
                                                     xr_all[:, s, co, :],
                                                     AF.Identity)

                    # finish BN2 coeffs (collective already in flight)
                    nc.scalar.dma_start(
                        ag2[:],
                        cc2o_d[:].rearrange("(r p) f -> p r f", p=P))
                    nc.vector.tensor_add(agt[:], ag2[:, 0, :], ag2[:, 1, :])
                    for rr in range(2, N_CORES):
                        nc.vector.tensor_add(agt[:], agt[:], ag2[:, rr, :])
                    nc.vector.tensor_scalar_mul(mtmp[:], agt[:, 0:CO],
                                                1.0 / NTOT)
                    nc.vector.tensor_scalar_mul(vtmp[:], agt[:, CO:2 * CO],
                                                1.0 / NTOT)
                    nc.vector.tensor_mul(ttmp[:], mtmp[:], mtmp[:])
                    nc.vector.tensor_sub(vtmp[:], vtmp[:], ttmp[:])
                    nc.scalar.activation(vtmp[:], vtmp[:], AF.Sqrt,
                                         bias=eps_t[:])
                    nc.vector.reciprocal(ttmp[:], vtmp[:])
                    nc.vector.tensor_mul(a2[:], g2[:], ttmp[:])
                    nc.vector.tensor_mul(ttmp[:], mtmp[:], a2[:])
                    nc.vector.tensor_sub(d2[:], be2[:], ttmp[:])

                    # fold BN2 into the MLP: W1a2 = W1*a2,
                    # b1p = b1 + W1 @ d2  (so the BN-apply pass disappears)
                    for ci in range(CO):
                        nc.vector.tensor_scalar(w1a2[:, ci, :],
                                                w1[:, ci, :],
                                                a2[:, ci:ci + 1], None,
                                                ALU.mult, ALU.bypass)
                    nc.vector.tensor_copy(d2b[:], d2[:])
                    cps2 = ppool.tile([P, 512], f32, tag="psC", bufs=1)
                    for mo in range(CO):
                        for ci in range(CO):
                            nc.tensor.matmul(cps2[:, mo:mo + 1],
                                             w1[:, ci, ts(mo, P)],
                                             d2b[:, ci, None],
                                             start=(ci == 0),
                                             stop=(ci == 3))
                    nc.vector.tensor_add(b1p[:], cps2[:, 0:CO], b1[:])

                    # ===== pass 3: out = xr + W2 relu(W1a2 xr + b1p) + b2
                    for s in range(B_LOC):
                        y1 = mpool.tile([P, CO, HW], bf16, tag="y1")
                        for mo in range(CO):
                            for n2 in range(2):
                                yps = ppool.tile([P, 512], f32, tag="ps512",
                                                 bufs=7)
                                for ci in range(CO):
                                    nc.tensor.matmul(
                                        yps[:],
                                        w1a2[:, ci, ts(mo, P)],
                                        xrb[:, s, ci, ts(n2, 512)],
                                        start=(ci == 0),
                                        stop=(ci == 3))
                                nc.scalar.activation(y1[:, mo, ts(n2, 512)],
                                                     yps[:], AF.Relu,
                                                     bias=b1p[:, mo:mo + 1])
                        ot = mpool.tile([P, CO, HW], f32, tag="ot")
                        for mo in range(CO):
                            for n2 in range(2):
                                yps = ppool.tile([P, 512], f32, tag="ps512",
                                                 bufs=7)
                                for ci in range(CO):
                                    nc.tensor.matmul(
                                        yps[:],
                                        w2[:, ci, ts(mo, P)],
                                        y1[:, ci, ts(n2, 512)],
                                        start=(ci == 0),
                                        stop=(ci == 3))
                                nc.vector.affine_then_add(
                                    out=ot[:, mo, ts(n2, 512)], in0=yps[:],
                                    in1=xr_all[:, s, mo, ts(n2, 512)],
                                    scale=1.0, bias=b2[:, mo:mo + 1])
                        for mo in range(CO):
                            nc.sync.dma_start(
                                chw_view(out_d, s)[:, mo:mo + 1, :],
                                ot[:, mo:mo + 1, :])

    nc.compile()
    return nc


def _prep_in_maps(inputs):
    import ml_dtypes
    bf = ml_dtypes.bfloat16
    f8 = ml_dtypes.float8_e4m3
    x = np.ascontiguousarray(inputs["x"], dtype=np.float32)
    wqkv = np.asarray(inputs["W_qkv"], dtype=np.float32)
    bqkv = np.asarray(inputs["b_qkv"], dtype=np.float32)

    def chan_t(w, dt=bf):  # [O, C] -> [P, CO, O]
        w = np.asarray(w, dtype=np.float32)
        o = w.shape[0]
        return np.ascontiguousarray(
            w.reshape(o, CO, P).transpose(2, 1, 0).astype(dt))

    def vec_t(v):  # [C] -> [P, CO]
        return np.ascontiguousarray(
            np.asarray(v, dtype=np.float32).reshape(CO, P).T)

    shared = {
        "wq_t": chan_t(np.concatenate([wqkv[:D], wqkv[:D]], axis=0), f8),
        "wk_t": chan_t(np.concatenate([wqkv[D:2 * D], wqkv[D:2 * D]],
                                      axis=0), f8),
        "wv_t": chan_t(wqkv[2 * D:], f8),
        "dwv_t": chan_t(wqkv[2 * D:]
                        - wqkv[2 * D:].astype(f8).astype(np.float32)),
        "w1_t": chan_t(inputs["W1"]),
        "w2_t": chan_t(inputs["W2"]),
        "bk_t": np.ascontiguousarray(
            np.concatenate([bqkv[D:2 * D], bqkv[D:2 * D]])[:, None],
            dtype=np.float32),
        "bv_t": vec_t(bqkv[2 * D:]),
        "b1_t": vec_t(inputs["b1"]),
        "b2_t": vec_t(inputs["b2"]),
        "g1_t": vec_t(inputs["bn1_g"]),
        "be1_t": vec_t(inputs["bn1_b"]),
        "g2_t": vec_t(inputs["bn2_g"]),
        "be2_t": vec_t(inputs["bn2_b"]),
        "ones_t": np.ones((P, P), dtype=bf),
    }
    in_maps = []
    for c in range(N_CORES):
        m = dict(shared)
        m["x"] = np.ascontiguousarray(
            x[c * B_LOC:(c + 1) * B_LOC].astype(bf))
        in_maps.append(m)
    return in_maps


def kernel_with_results(inputs, trace=False):
    from concourse import bass_utils
    if "nc" not in _CACHE:
        _CACHE["nc"] = _build_nc()
    nc = _CACHE["nc"]
    in_maps = _prep_in_maps(inputs)
    res = bass_utils.run_bass_kernel_spmd(
        nc, in_maps, core_ids=list(range(N_CORES)), trace=trace)
    out = np.concatenate([res.results[c]["out"] for c in range(N_CORES)],
                         axis=0)
    return out, res


def kernel(**inputs):
    out, _ = kernel_with_results(inputs, trace=False)
    return out


# revision 30
# speedup vs baseline: 1.1130x; 1.1130x over previous
"""Trainium2 Bass kernel for nn_AttentionLayer (B=32, C=512, HW=1024).

Data-parallel over batch across 8 NeuronCores (4 samples each).  BN batch
stats are per-core partials + a tiny 8-core AllGather, twice (BN1 on x,
BN2 on xr = x + attention).  Attention matmuls run on TensorE in fp8
DoubleRow (fp8 Wv rounding error cancelled by a per-channel bias
dWv@mean(h), exploiting sum_q softmax == 1); the MLP runs in bf16 with
BN2 folded into W1; the residual path stays f32 and xr stays resident in
SBUF (no DRAM round-trip).  x is loaded in bf16.  BN2 stats come from
fused accumulators on the residual writes.  Dummy matmuls keep the PE
HAM clock un-throttled through the x-load and collective windows.

kernel(**inputs) takes FULL unsharded inputs, returns the FULL output.
"""

import numpy as np

B, C, HW = 32, 512, 1024
D = C // 8            # 64
N_CORES = 8
B_LOC = B // N_CORES  # 4
P = 128
CO = C // P           # 4
NTOT = float(B * HW)  # BN normalizer (biased stats over batch+spatial)
EPS = 1e-5

# PE clock-warm heartbeat (dummy matmuls, ~0.21-0.43us each)
WARM_A = 135   # covers x-load + BN1 AllGather window
WARM_2 = 50    # covers BN2 AllGather window

_CACHE = {}


def _build_nc():
    import concourse.bass as bass
    import concourse.mybir as mybir
    import concourse.tile as tile
    from concourse import bacc
    from concourse.bass import ts

    f32 = mybir.dt.float32
    bf16 = mybir.dt.bfloat16
    f8 = mybir.dt.float8e4
    PM = mybir.MatmulPerfMode
    AF = mybir.ActivationFunctionType
    ALU = mybir.AluOpType
    AX = mybir.AxisListType

    nc = bacc.Bacc("TRN2", target_bir_lowering=False, debug=False,
                   num_devices=N_CORES)

    # ---------------- I/O ----------------
    x_d = nc.dram_tensor("x", [B_LOC, C, HW], bf16, kind="ExternalInput")
    wq_d = nc.dram_tensor("wq_t", [P, CO, P], f8, kind="ExternalInput")
    wk_d = nc.dram_tensor("wk_t", [P, CO, P], f8, kind="ExternalInput")
    wv_d = nc.dram_tensor("wv_t", [P, CO, C], f8, kind="ExternalInput")
    dwv_d = nc.dram_tensor("dwv_t", [P, CO, C], bf16, kind="ExternalInput")
    w1_d = nc.dram_tensor("w1_t", [P, CO, C], bf16, kind="ExternalInput")
    w2_d = nc.dram_tensor("w2_t", [P, CO, C], bf16, kind="ExternalInput")
    bk_d = nc.dram_tensor("bk_t", [P, 1], f32, kind="ExternalInput")
    bv_d = nc.dram_tensor("bv_t", [P, CO], f32, kind="ExternalInput")
    b1_d = nc.dram_tensor("b1_t", [P, CO], f32, kind="ExternalInput")
    b2_d = nc.dram_tensor("b2_t", [P, CO], f32, kind="ExternalInput")
    g1_d = nc.dram_tensor("g1_t", [P, CO], f32, kind="ExternalInput")
    be1_d = nc.dram_tensor("be1_t", [P, CO], f32, kind="ExternalInput")
    g2_d = nc.dram_tensor("g2_t", [P, CO], f32, kind="ExternalInput")
    be2_d = nc.dram_tensor("be2_t", [P, CO], f32, kind="ExternalInput")
    ones_d = nc.dram_tensor("ones_t", [P, P], bf16, kind="ExternalInput")
    out_d = nc.dram_tensor("out", [B_LOC, C, HW], f32, kind="ExternalOutput")

    def chw_view(dram3, s):
        # [C, HW] sample -> [P, CO, HW] partition view (c = co*P + p)
        return dram3[s].rearrange("(co p) hw -> p co hw", p=P)

    with tile.TileContext(nc) as tc:
        with (
            tc.tile_pool(name="const", bufs=1) as cpool,
            tc.tile_pool(name="stats", bufs=1) as spool,
            tc.tile_pool(name="dram", bufs=1, space="DRAM") as dpool,
            tc.tile_pool(name="psum", bufs=1, space="PSUM") as ppool,
        ):
            # ---------- persistent weights ----------
            wq = cpool.tile([P, CO, P], f8)
            wk = cpool.tile([P, CO, P], f8)
            wv = cpool.tile([P, CO, C], f8)
            dwv = cpool.tile([P, CO, C], bf16)
            w1 = cpool.tile([P, CO, C], bf16)
            w1a2 = cpool.tile([P, CO, C], bf16)   # W1 * a2 (BN2 folded)
            w2 = cpool.tile([P, CO, C], bf16)
            bk = cpool.tile([P, 1], f32)
            bv = cpool.tile([P, CO], f32)
            b1 = cpool.tile([P, CO], f32)
            b1p = cpool.tile([P, CO], f32)        # b1 + W1 @ d2
            b2 = cpool.tile([P, CO], f32)
            g1 = cpool.tile([P, CO], f32)
            be1 = cpool.tile([P, CO], f32)
            g2 = cpool.tile([P, CO], f32)
            be2 = cpool.tile([P, CO], f32)
            d2b = cpool.tile([P, CO], bf16)
            ones128 = cpool.tile([P, P], bf16)
            dummy = cpool.tile([P, 512], bf16)
            eps_t = cpool.tile([P, 1], f32)
            nc.gpsimd.memset(eps_t[:], EPS)
            nc.gpsimd.memset(dummy[:], 0.5)

            # ---------- stats tiles ----------
            ssum1 = spool.tile([P, CO, B_LOC], f32)
            ssq1 = spool.tile([P, CO, B_LOC], f32)
            ssum2 = spool.tile([P, CO, B_LOC], f32)
            s2acc = spool.tile([P, CO, 2 * B_LOC], f32)   # attsum per (mo,n2)
            q2acc = spool.tile([P, CO, 2 * B_LOC], f32)   # sum(xr^2) per (mo,n2)
            ccin1 = spool.tile([P, 2 * CO], f32)
            ccin2 = spool.tile([P, 2 * CO], f32)
            ag1 = spool.tile([P, N_CORES, 2 * CO], f32)
            ag2 = spool.tile([P, N_CORES, 2 * CO], f32)
            a1 = spool.tile([P, CO], f32)
            d1 = spool.tile([P, CO], f32)
            a2 = spool.tile([P, CO], f32)
            d2 = spool.tile([P, CO], f32)
            mtmp = spool.tile([P, CO], f32)
            vtmp = spool.tile([P, CO], f32)
            ttmp = spool.tile([P, CO], f32)
            agt = spool.tile([P, 2 * CO], f32)
            junk1 = spool.tile([P, 1], f32)

            # DRAM scratch (collective in/out only)
            cc1i_d = dpool.tile([P, 2 * CO], f32)
            cc1o_d = dpool.tile([N_CORES * P, 2 * CO], f32)
            cc2i_d = dpool.tile([P, 2 * CO], f32)
            cc2o_d = dpool.tile([N_CORES * P, 2 * CO], f32)

            def heartbeat(n):
                """Dummy matmuls keeping the PE HAM clock at 8/8 through
                windows where real matmuls are blocked on collectives."""
                if n <= 0:
                    return
                wt = ppool.tile([P, 512], f32, tag="psC", bufs=1)
                for _ in range(n):
                    nc.tensor.matmul(wt[:], dummy[:, 0:P], dummy[:],
                                     start=True, stop=True)

            def bn_coeffs(cci_d, cco_d, ag_sb, gg, bb, aa, dd):
                """AllGather -> local sum -> a = g*rsqrt(var+eps),
                d = b - mean*a"""
                nc.gpsimd.collective_compute(
                    "AllGather", ALU.bypass,
                    replica_groups=[list(range(N_CORES))],
                    ins=[cci_d[:].opt()], outs=[cco_d[:].opt()],
                )
                nc.scalar.dma_start(
                    ag_sb[:],
                    cco_d[:].rearrange("(r p) f -> p r f", p=P))
                nc.vector.tensor_add(agt[:], ag_sb[:, 0, :], ag_sb[:, 1, :])
                for rr in range(2, N_CORES):
                    nc.vector.tensor_add(agt[:], agt[:], ag_sb[:, rr, :])
                nc.vector.tensor_scalar_mul(mtmp[:], agt[:, 0:CO],
                                            1.0 / NTOT)
                nc.vector.tensor_scalar_mul(vtmp[:], agt[:, CO:2 * CO],
                                            1.0 / NTOT)
                nc.vector.tensor_mul(ttmp[:], mtmp[:], mtmp[:])
                nc.vector.tensor_sub(vtmp[:], vtmp[:], ttmp[:])
                nc.scalar.activation(vtmp[:], vtmp[:], AF.Sqrt, bias=eps_t[:])
                nc.vector.reciprocal(ttmp[:], vtmp[:])
                nc.vector.tensor_mul(aa[:], gg[:], ttmp[:])
                nc.vector.tensor_mul(ttmp[:], mtmp[:], aa[:])
                nc.vector.tensor_sub(dd[:], bb[:], ttmp[:])

            with tc.tile_pool(name="xrp", bufs=1) as xrpool:
                xr_all = xrpool.tile([P, B_LOC, CO, HW], f32)

                with tc.tile_pool(name="xp", bufs=1) as xpool:
                    x_all = xpool.tile([P, B_LOC, CO, HW], bf16)

                    # PE heartbeat through x-load + AG1 (independent ops,
                    # run back-to-back from t~0)
                    heartbeat(WARM_A)

                    # ============ pass 1: x load + BN1 stats ============
                    # 32 reduction passes spread across DVE/ACT/Pool so no
                    # single engine trails the DMA stream
                    with tc.tile_pool(name="p1", bufs=2) as w1pool:
                        for s in range(B_LOC):
                            for co in range(CO):
                                nc.sync.dma_start(
                                    x_all[:, s, co:co + 1, :],
                                    chw_view(x_d, s)[:, co:co + 1, :])
                                i = 4 * s + co
                                xt_a = x_all[:, s, co, :]
                                # sums: 12 DVE, 4 ACT / squares: 12 ACT,
                                # 4 DVE -> 16 passes each engine
                                sq = w1pool.tile([P, HW], bf16, tag="sq1")
                                if i % 4 == 3:
                                    pj = w1pool.tile([P, HW], bf16,
                                                     tag="pj1")
                                    nc.scalar.activation(
                                        pj[:], xt_a, AF.Identity,
                                        accum_out=ssum1[:, co, s:s + 1])
                                    nc.vector.affine_mul_reduce(
                                        out=sq[:],
                                        accum_out=ssq1[:, co, s:s + 1],
                                        in0=xt_a, in1=xt_a,
                                        scale=1.0, bias=0.0)
                                else:
                                    nc.vector.tensor_reduce(
                                        ssum1[:, co, s:s + 1], xt_a,
                                        axis=AX.X, op=ALU.add)
                                    nc.scalar.activation(
                                        sq[:], xt_a, AF.Square,
                                        accum_out=ssq1[:, co, s:s + 1])

                    # weight/bias loads (issued after the x DMAs on purpose)
                    for t, d in [(wq, wq_d), (wk, wk_d), (wv, wv_d),
                                 (dwv, dwv_d), (w1, w1_d),
                                 (w2, w2_d), (bk, bk_d), (bv, bv_d),
                                 (b1, b1_d), (b2, b2_d), (g1, g1_d),
                                 (be1, be1_d), (g2, g2_d), (be2, be2_d),
                                 (ones128, ones_d)]:
                        nc.sync.dma_start(t[:], d[:])

                    nc.vector.tensor_reduce(ccin1[:, 0:CO, None], ssum1[:],
                                            axis=AX.X, op=ALU.add)
                    nc.scalar.dma_start(cc1i_d[:, 0:CO], ccin1[:, 0:CO])
                    nc.vector.tensor_reduce(ccin1[:, CO:2 * CO, None],
                                            ssq1[:], axis=AX.X, op=ALU.add)
                    nc.scalar.dma_start(cc1i_d[:, CO:2 * CO],
                                        ccin1[:, CO:2 * CO])
                    bn_coeffs(cc1i_d, cc1o_d, ag1, g1, be1, a1, d1)

                    # ======== pass 2: attention, xr = x + att ========
                    with tc.tile_pool(name="p2b", bufs=2) as bpool:
                        for s in range(B_LOC):
                            if s > 0:
                                # cover the PE idle window while ACT
                                # computes this sample's h (HAM would
                                # re-throttle after ~3.4us idle)
                                heartbeat(8)
                            xt = x_all[:, s]
                            qz = bpool.tile([P, HW], bf16, tag="qz")
                            kz = bpool.tile([P, HW], bf16, tag="kz")

                            # h = relu(a1*x + d1); hsum rows for the fp8-Wv
                            # DC correction (sum_q E/Z == 1 exactly)
                            h = bpool.tile([P, CO, HW], f8, tag="h", bufs=2)
                            hsum = bpool.tile([P, CO], f32, tag="hsum")
                            for co in range(CO):
                                nc.scalar.activation(
                                    h[:, co, :], xt[:, co, :], AF.Relu,
                                    bias=d1[:, co:co + 1],
                                    scale=a1[:, co:co + 1],
                                    accum_out=hsum[:, co:co + 1])
                            # q = Wq @ h (bias dropped: constant-per-column
                            # terms cancel in softmax over q), k = Wk @ h +
                            # bk; each duplicated into both partition halves
                            # so the beta matmuls can row-pack two K=64 tiles
                            for n2 in range(2):
                                qps = ppool.tile([P, 512], f32, tag="ps512",
                                                 bufs=7)
                                for c2 in range(2):
                                    nc.tensor.matmul(
                                        qps[:],
                                        wq[:, 2 * c2:2 * c2 + 2, :],
                                        h[:, 2 * c2:2 * c2 + 2, ts(n2, 512)],
                                        start=(c2 == 0), stop=(c2 == 1),
                                        perf_mode=PM.DoubleRow)
                                nc.scalar.activation(qz[:, ts(n2, 512)],
                                                     qps[:], AF.Identity)
                                kps = ppool.tile([P, 512], f32, tag="ps512",
                                                 bufs=7)
                                for c2 in range(2):
                                    nc.tensor.matmul(
                                        kps[:],
                                        wk[:, 2 * c2:2 * c2 + 2, :],
                                        h[:, 2 * c2:2 * c2 + 2, ts(n2, 512)],
                                        start=(c2 == 0), stop=(c2 == 1),
                                        perf_mode=PM.DoubleRow)
                                nc.scalar.activation(kz[:, ts(n2, 512)],
                                                     kps[:], AF.Identity,
                                                     bias=bk[:])

                            # vT[hw, c] = h^T @ Wv^T (bv folded into xr)
                            vt = bpool.tile([P, 8, C], f8, tag="vt", bufs=2)
                            for jw in range(8):
                                vtps = ppool.tile([P, 512], f32, tag="ps512",
                                                  bufs=7)
                                for c2 in range(2):
                                    nc.tensor.matmul(
                                        vtps[:],
                                        h[:, 2 * c2:2 * c2 + 2, ts(jw, P)],
                                        wv[:, 2 * c2:2 * c2 + 2, :],
                                        start=(c2 == 0), stop=(c2 == 1),
                                        perf_mode=PM.DoubleRow)
                                if jw % 2 == 0:
                                    nc.vector.tensor_copy(vt[:, jw, :],
                                                          vtps[:])
                                else:
                                    nc.scalar.activation(vt[:, jw, :],
                                                         vtps[:],
                                                         AF.Identity)

                            # E = exp(q^T k / 8) in [q, k] layout, with a
                            # bf16 tree presum for Z on the idle Pool engine
                            E = bpool.tile([P, 8, HW], f8, tag="E", bufs=2)
                            et = bpool.tile([P, 4, HW], bf16, tag="et",
                                            bufs=1)
                            lo, hi = slice(0, D), slice(D, P)
                            for j2 in range(4):
                                je, jo = 2 * j2, 2 * j2 + 1
                                bps = {}
                                for n2 in range(2):
                                    be = ppool.tile([P, 512], f32,
                                                    tag="ps512", bufs=7)
                                    bo = ppool.tile([P, 512], f32,
                                                    tag="ps512", bufs=7)
                                    nc.tensor.matmul(be[:],
                                                     qz[lo, ts(je, P)],
                                                     kz[lo, ts(n2, 512)],
                                                     start=True, stop=True)
                                    nc.tensor.matmul(bo[:],
                                                     qz[hi, ts(jo, P)],
                                                     kz[hi, ts(n2, 512)],
                                                     start=True, stop=True)
                                    bps[n2] = (be, bo)
                                for n2 in range(2):
                                    be, bo = bps[n2]
                                    nc.scalar.activation(
                                        E[:, je, ts(n2, 512)],
                                        be[:], AF.Exp, scale=0.125)
                                    nc.scalar.activation(
                                        E[:, jo, ts(n2, 512)],
                                        bo[:], AF.Exp, scale=0.125)
                                nc.vector.tensor_add(et[:, j2, :],
                                                     E[:, je, :],
                                                     E[:, jo, :])

                            # fp8-Wv DC correction bias
                            hm = bpool.tile([P, CO], bf16, tag="hm")
                            nc.vector.tensor_scalar_mul(hm[:], hsum[:],
                                                        1.0 / HW)
                            cps = ppool.tile([P, 512], f32, tag="psC",
                                             bufs=1)
                            for mo in range(CO):
                                for ci in range(CO):
                                    nc.tensor.matmul(cps[:, mo:mo + 1],
                                                     dwv[:, ci, ts(mo, P)],
                                                     hm[:, ci, None],
                                                     start=(ci == 0),
                                                     stop=(ci == 3))
                            biasn = bpool.tile([P, CO], f32, tag="biasn")
                            nc.vector.tensor_add(biasn[:], cps[:, 0:CO],
                                                 bv[:])

                            # att = (v @ E) / Z ; xr = x + att + bias
                            aps_tiles = {}
                            rz = bpool.tile([P, HW], f32, tag="rz")

                            # Z partition-reduce + reciprocal (before the
                            # att groups so rz never gates a consume)
                            for n2 in range(2):
                                zps = ppool.tile([P, 512], f32,
                                                 tag="ps512", bufs=7)
                                for j2 in range(4):
                                    nc.tensor.matmul(
                                        zps[:],
                                        ones128[:],
                                        et[:, j2, ts(n2, 512)],
                                        start=(j2 == 0),
                                        stop=(j2 == 3))
                                nc.vector.reciprocal_approx_fast(
                                    out=rz[:, ts(n2, 512)],
                                    in_=zps[:])

                            def att_group(mo, n2):
                                aps = ppool.tile([P, 512], f32, tag="ps512",
                                                 bufs=7)
                                for j4 in range(4):
                                    nc.tensor.matmul(
                                        aps[:],
                                        vt[:, 2 * j4:2 * j4 + 2, ts(mo, P)],
                                        E[:, 2 * j4:2 * j4 + 2, ts(n2, 512)],
                                        start=(j4 == 0), stop=(j4 == 3),
                                        perf_mode=PM.DoubleRow)
                                aps_tiles[(mo, n2)] = aps

                            last_s = (s == B_LOC - 1)

                            def consume(mo, n2):
                                aps = aps_tiles.pop((mo, n2))
                                dst = xr_all[:, s, mo, ts(n2, 512)]
                                i2 = 2 * s + n2
                                # att = aps*rz (accum: attsum), then
                                # xr = att + biasn + x, then sumsq accum
                                # via a square affine_mul_reduce
                                nc.vector.affine_mul_reduce(
                                    out=dst,
                                    accum_out=s2acc[:, mo, i2:i2 + 1],
                                    in0=aps[:], in1=rz[:, ts(n2, 512)],
                                    scale=1.0, bias=0.0)
                                nc.vector.affine_then_add(
                                    out=dst, in0=dst,
                                    in1=xt[:, mo, ts(n2, 512)],
                                    scale=1.0, bias=biasn[:, mo:mo + 1])
                                sqj = bpool.tile([P, 512], bf16, tag="sqj",
                                                 bufs=2)
                                nc.vector.affine_mul_reduce(
                                    out=sqj[:],
                                    accum_out=q2acc[:, mo, i2:i2 + 1],
                                    in0=dst, in1=dst,
                                    scale=1.0, bias=0.0)

                            groups = [(mo, n2) for mo in range(CO)
                                      for n2 in range(2)]
                            for idx, g in enumerate(groups):
                                att_group(*g)
                                lag = 2 if last_s else 5
                                if idx >= lag:
                                    consume(*groups[idx - lag])
                            for g in groups[-(2 if last_s else 5):]:
                                consume(*g)

                            # ssum2[:, :, s] = sum_hw(x) + attsum + HW*bias
                            atot = bpool.tile([P, CO], f32, tag="atot")
                            nc.vector.tensor_reduce(
                                atot[:, :, None],
                                s2acc[:, :, 2 * s:2 * s + 2],
                                axis=AX.X, op=ALU.add)
                            nc.vector.tensor_add(atot[:], atot[:],
                                                 ssum1[:, :, s])
                            nc.vector.tensor_scalar(ssum2[:, :, s],
                                                    biasn[:],
                                                    float(HW), None,
                                                    ALU.mult, ALU.bypass)
                            nc.vector.tensor_add(ssum2[:, :, s],
                                                 ssum2[:, :, s], atot[:])

                        # pack BN2 partials -> AG2
                        nc.vector.tensor_reduce(ccin2[:, 0:CO, None],
                                                ssum2[:], axis=AX.X,
                                                op=ALU.add)
                        nc.scalar.dma_start(cc2i_d[:, 0:CO],
                                            ccin2[:, 0:CO])
                        nc.vector.tensor_reduce(ccin2[:, CO:2 * CO, None],
                                                q2acc[:], axis=AX.X,
                                                op=ALU.add)
                        nc.scalar.dma_start(cc2i_d[:, CO:2 * CO],
                                            ccin2[:, CO:2 * CO])
                        nc.gpsimd.collective_compute(
                            "AllGather", ALU.bypass,
                            replica_groups=[list(range(N_CORES))],
                            ins=[cc2i_d[:].opt()], outs=[cc2o_d[:].opt()],
                        )

                # ===== gap3: keep PE warm; cast xr->bf16 during AG2 =====
                heartbeat(WARM_2)

                with tc.tile_pool(name="mp", bufs=2) as mpool:
                    xrb = mpool.tile([P, B_LOC, CO, HW], bf16, tag="xrb",
                                     bufs=1)
                    for s in range(B_LOC):
                        for co in range(CO):
                            if (4 * s + co) % 2 == 0:
                                nc.vector.tensor_copy(xrb[:, s, co, :],
                                                      xr_all[:, s, co, :])
                            else:
                                nc.scalar.activation(xrb[:, s, co, :],
                                                     xr_all[:, s, co, :],
                                                     AF.Identity)

                    # finish BN2 coeffs (collective already in flight)
                    nc.scalar.dma_start(
                        ag2[:],
                        cc2o_d[:].rearrange("(r p) f -> p r f", p=P))
                    nc.vector.tensor_add(agt[:], ag2[:, 0, :], ag2[:, 1, :])
                    for rr in range(2, N_CORES):
                        nc.vector.tensor_add(agt[:], agt[:], ag2[:, rr, :])
                    nc.vector.tensor_scalar_mul(mtmp[:], agt[:, 0:CO],
                                                1.0 / NTOT)
                    nc.vector.tensor_scalar_mul(vtmp[:], agt[:, CO:2 * CO],
                                                1.0 / NTOT)
                    nc.vector.tensor_mul(ttmp[:], mtmp[:], mtmp[:])
                    nc.vector.tensor_sub(vtmp[:], vtmp[:], ttmp[:])
                    nc.scalar.activation(vtmp[:], vtmp[:], AF.Sqrt,
                                         bias=eps_t[:])
                    nc.vector.reciprocal(ttmp[:], vtmp[:])
                    nc.vector.tensor_mul(a2[:], g2[:], ttmp[:])
                    nc.vector.tensor_mul(ttmp[:], mtmp[:], a2[:])
                    nc.vector.tensor_sub(d2[:], be2[:], ttmp[:])

                    # fold BN2 into the MLP: W1a2 = W1*a2,
                    # b1p = b1 + W1 @ d2  (so the BN-apply pass disappears)
                    for ci in range(CO):
                        nc.vector.tensor_scalar(w1a2[:, ci, :],
                                                w1[:, ci, :],
                                                a2[:, ci:ci + 1], None,
                                                ALU.mult, ALU.bypass)
                    nc.vector.tensor_copy(d2b[:], d2[:])
                    cps2 = ppool.tile([P, 512], f32, tag="psC", bufs=1)
                    for mo in range(CO):
                        for ci in range(CO):
                            nc.tensor.matmul(cps2[:, mo:mo + 1],
                                             w1[:, ci, ts(mo, P)],
                                             d2b[:, ci, None],
                                             start=(ci == 0),
                                             stop=(ci == 3))
                    nc.vector.tensor_add(b1p[:], cps2[:, 0:CO], b1[:])

                    # ===== pass 3: out = xr + W2 relu(W1a2 xr + b1p) + b2
                    for s in range(B_LOC):
                        y1 = mpool.tile([P, CO, HW], bf16, tag="y1")
                        for mo in range(CO):
                            for n2 in range(2):
                                yps = ppool.tile([P, 512], f32, tag="ps512",
                                                 bufs=7)
                                for ci in range(CO):
                                    nc.tensor.matmul(
                                        yps[:],
                                        w1a2[:, ci, ts(mo, P)],
                                        xrb[:, s, ci, ts(n2, 512)],
                                        start=(ci == 0),
                                        stop=(ci == 3))
                                nc.scalar.activation(y1[:, mo, ts(n2, 512)],
                                                     yps[:], AF.Relu,
                                                     bias=b1p[:, mo:mo + 1])
                        ot = mpool.tile([P, CO, HW], f32, tag="ot")
                        for mo in range(CO):
                            for n2 in range(2):
                                yps = ppool.tile([P, 512], f32, tag="ps512",
                                                 bufs=7)
                                for ci in range(CO):
                                    nc.tensor.matmul(
                                        yps[:],
                                        w2[:, ci, ts(mo, P)],
                                        y1[:, ci, ts(n2, 512)],
                                        start=(ci == 0),
                                        stop=(ci == 3))
                                nc.vector.affine_then_add(
                                    out=ot[:, mo, ts(n2, 512)], in0=yps[:],
                                    in1=xr_all[:, s, mo, ts(n2, 512)],
                                    scale=1.0, bias=b2[:, mo:mo + 1])
                        for mo in range(CO):
                            nc.sync.dma_start(
                                chw_view(out_d, s)[:, mo:mo + 1, :],
                                ot[:, mo:mo + 1, :])

    nc.compile()
    return nc


def _prep_in_maps(inputs):
    import ml_dtypes
    bf = ml_dtypes.bfloat16
    f8 = ml_dtypes.float8_e4m3
    x = np.ascontiguousarray(inputs["x"], dtype=np.float32)
    wqkv = np.asarray(inputs["W_qkv"], dtype=np.float32)
    bqkv = np.asarray(inputs["b_qkv"], dtype=np.float32)

    def chan_t(w, dt=bf):  # [O, C] -> [P, CO, O]
        w = np.asarray(w, dtype=np.float32)
        o = w.shape[0]
        return np.ascontiguousarray(
            w.reshape(o, CO, P).transpose(2, 1, 0).astype(dt))

    def vec_t(v):  # [C] -> [P, CO]
        return np.ascontiguousarray(
            np.asarray(v, dtype=np.float32).reshape(CO, P).T)

    shared = {
        "wq_t": chan_t(np.concatenate([wqkv[:D], wqkv[:D]], axis=0), f8),
        "wk_t": chan_t(np.concatenate([wqkv[D:2 * D], wqkv[D:2 * D]],
                                      axis=0), f8),
        "wv_t": chan_t(wqkv[2 * D:], f8),
        "dwv_t": chan_t(wqkv[2 * D:]
                        - wqkv[2 * D:].astype(f8).astype(np.float32)),
        "w1_t": chan_t(inputs["W1"]),
        "w2_t": chan_t(inputs["W2"]),
        "bk_t": np.ascontiguousarray(
            np.concatenate([bqkv[D:2 * D], bqkv[D:2 * D]])[:, None],
            dtype=np.float32),
        "bv_t": vec_t(bqkv[2 * D:]),
        "b1_t": vec_t(inputs["b1"]),
        "b2_t": vec_t(inputs["b2"]),
        "g1_t": vec_t(inputs["bn1_g"]),
        "be1_t": vec_t(inputs["bn1_b"]),
        "g2_t": vec_t(inputs["bn2_g"]),
        "be2_t": vec_t(inputs["bn2_b"]),
        "ones_t": np.ones((P, P), dtype=bf),
    }
    in_maps = []
    for c in range(N_CORES):
        m = dict(shared)
        m["x"] = np.ascontiguousarray(
            x[c * B_LOC:(c + 1) * B_LOC].astype(bf))
        in_maps.append(m)
    return in_maps


def kernel_with_results(inputs, trace=False):
    from concourse import bass_utils
    if "nc" not in _CACHE:
        _CACHE["nc"] = _build_nc()
    nc = _CACHE["nc"]
    in_maps = _prep_in_maps(inputs)
    res = bass_utils.run_bass_kernel_spmd(
        nc, in_maps, core_ids=list(range(N_CORES)), trace=trace)
    out = np.concatenate([res.results[c]["out"] for c in range(N_CORES)],
                         axis=0)
    return out, res


def kernel(**inputs):
    out, _ = kernel_with_results(inputs, trace=False)
    return out
